# revision 27
# baseline (speedup 1.0000x reference)
"""Single-head causal attention on 8 TRN2 NeuronCores, batch-parallel.

Problem: x[8,2048,1024] f32, Wq/Wk/Wv[1024,64] f32
  q,k,v = x@W*  ;  scores = q k^T / sqrt(1024), causal  ;  out = softmax(scores) @ v

Sharding: batch dim across 8 cores (1 batch element per core, no collectives).

v5 dataflow (bf16 datapath, PE transposes, chunk waves):
  - host passes x/W*/tri/ident in bf16 (tolerance 2e-2 >> bf16's ~4e-3);
    weights pre-packed [c%128, c//128, h] so every DMA is wide + contiguous.
  - x arrives in 16 big per-tile DMAs ([128,1024] bf16, 256KB each), reverse
    chunk order so the last-arriving chunk gates the fewest score tiles.
  - per tile: 8 PE transposes (bf16, 1 cyc/row) into ONE [128,2,512] bf16 psum
    bank -> one 2x-rate DVE copy -> xT bf16.  Keeps PE continuously busy
    (p-state stays at max) and DMA count low (sem recycling stays off the
    critical path).
  - per chunk: qk-proj ([128,512] psum: rows 0:64 q, 64:128 k -> qkT bf16, one
    copy); kT to partitions 0:64 via Pool/SWDGE SBUF->SBUF DMA; v-proj natural
    [t,64] per tile.
  - scores: st[s,t] [128,512] tiles, lhsT=kT rhs=qT (bf16); exp on Act in
    PAIRS ([128,2,512] psum -> wst bf16), diag pairs column-trimmed; diagonal
    tri-masks batched on Pool after the waves.
  - out natural: po[t,h] = sum_j wst_j^T @ [v_j | 1]; col 64 = softmax denom;
    epilogue reciprocal + tensor_scalar_mul (DVE); out f32 DMA per chunk (Act).
"""

import numpy as np

import concourse.bacc as bacc
import concourse.mybir as mybir
import concourse.tile as tile
from concourse.bass_utils import run_bass_kernel_spmd

F32 = mybir.dt.float32
BF16 = mybir.dt.bfloat16
EXP = mybir.ActivationFunctionType.Exp

B, T, C, H = 8, 2048, 1024, 64
NCT = C // 128          # 8 c-tiles
NTT = T // 128          # 16 t-tiles
SCALE = float(C ** -0.5)

WAVES = [3, 2, 1, 0]    # chunk per wave, reverse order
ST_WAVE = [
    [(3, 12), (3, 13), (3, 14), (3, 15)],
    [(2, 8), (2, 9), (2, 10), (2, 11), (3, 8), (3, 9), (3, 10), (3, 11)],
    [(1, 4), (1, 5), (1, 6), (1, 7), (3, 4), (3, 5), (3, 6), (3, 7),
     (2, 4), (2, 5), (2, 6), (2, 7)],
    [(0, 0), (0, 1), (0, 2), (0, 3), (3, 0), (3, 1), (3, 2), (3, 3),
     (2, 0), (2, 1), (2, 2), (2, 3), (1, 0), (1, 1), (1, 2), (1, 3)],
]
AVAIL = [12, 13, 14, 15, 8, 9, 10, 11, 4, 5, 6, 7, 0, 1, 2, 3]

_CACHE = {}


def build():
    nc = bacc.Bacc(name="head_attn")
    xb_d = nc.dram_tensor("xb", [T, C], BF16, kind="ExternalInput")
    wqk_d = nc.dram_tensor("wqkb", [128, NCT, 128], BF16, kind="ExternalInput")
    wv_d = nc.dram_tensor("wvb", [128, NCT, H], BF16, kind="ExternalInput")
    tri_d = nc.dram_tensor("trib", [128, 128], BF16, kind="ExternalInput")
    id_d = nc.dram_tensor("identb", [128, 128], BF16, kind="ExternalInput")
    out_d = nc.dram_tensor("out", [T, H], F32, kind="ExternalOutput")

    with tile.TileContext(nc) as tc:
        with (
            tc.tile_pool(name="singles", bufs=1) as singles,
            tc.tile_pool(name="xstage", bufs=4) as xstage,
            tc.tile_pool(name="wstp", bufs=1) as wstp,
            tc.tile_pool(name="outp", bufs=1) as outp,
            tc.tile_pool(name="recp", bufs=2) as recp,
            tc.tile_pool(name="ptr", bufs=2, space="PSUM") as ptr,
            tc.tile_pool(name="pqv", bufs=2, space="PSUM") as pqv,
            tc.tile_pool(name="pst", bufs=2, space="PSUM") as pst,
        ):
            # ---- constants / weights (SP queue first, tiny + wide transfers)
            identb = singles.tile([128, 128], BF16)
            nc.gpsimd.dma_start(identb, id_d[:, :])
            wqkb = singles.tile([128, NCT, 128], BF16)
            wvb = singles.tile([128, NCT, H], BF16)
            trib = singles.tile([128, 128], BF16)
            nc.gpsimd.dma_start(wqkb, wqk_d[:, :, :])
            nc.gpsimd.dma_start(wvb, wv_d[:, :, :])

            xT = singles.tile([128, NCT, T], BF16)      # [c, ct, t]
            qkT = singles.tile([128, T], BF16)          # rows 0:64 qT, 64:128 kT
            kTsb = singles.tile([64, T], BF16)          # kT at base partition 0
            v_sb = singles.tile([128, NTT, 66], BF16)   # v natural + ones col 64

            out_v = out_d.rearrange("(c a p) h -> c p a h", a=4, p=128)

            wst_loc = {}
            exp_order = []
            n_exp = [0]

            def flush_pairs(pairs):
                for g in pairs:
                    ks = [j - 4 * i for (i, j) in g]
                    d = 128 * min(ks) if all(k >= 0 for k in ks) else 0
                    pt = pst.tile([128, 2, 512], F32, tag="pair", name="pt")
                    for h, (i, j) in enumerate(g):
                        nc.tensor.matmul(pt[:, h, d:],
                                         kTsb[:, j * 128:(j + 1) * 128],
                                         qkT[0:64, i * 512 + d:(i + 1) * 512],
                                         start=True, stop=True)
                    wt = wstp.tile([128, 2, 512], BF16, tag=f"w{n_exp[0]}",
                                   name="wt")
                    n_exp[0] += 1
                    nc.scalar.activation(wt[:, 0:len(g), d:], pt[:, 0:len(g), d:],
                                         EXP, scale=SCALE)
                    for h, (i, j) in enumerate(g):
                        wst_loc[(i, j)] = (wt, h)
                        exp_order.append((i, j))

            # ---- wave loop (one chunk per wave)
            for w, c in enumerate(WAVES):
                if w == 1:
                    # deferred: needed only by the post-wave tri batch / po,
                    # keeps the Pool queue clear ahead of wave 0's kT DMA
                    nc.gpsimd.dma_start(trib, tri_d[:, :])
                    nc.gpsimd.memset(v_sb[:, :, 64:66], 1.0)
                t0, t1 = c * 512, (c + 1) * 512
                xs_tiles = []
                for tt in range(4 * c, 4 * c + 4):
                    xs = xstage.tile([128, C], BF16, tag="xs", name="xs")
                    nc.sync.dma_start(xs, xb_d[tt * 128:(tt + 1) * 128, :])
                    xs_tiles.append((tt, xs))

                # transpose each tile into xT (bf16 PE transposes, one
                # [128,2,512] bf16 psum bank, one 2x-rate DVE copy)
                for tt, xs in xs_tiles:
                    pt = ptr.tile([128, 2, 512], BF16, tag="tr", name="ptr")
                    for ct in range(NCT):
                        nc.tensor.transpose(
                            pt[:, ct // 4, (ct % 4) * 128:(ct % 4 + 1) * 128],
                            xs[:, ct * 128:(ct + 1) * 128], identb)
                    nc.vector.tensor_copy(
                        xT[:, :, tt * 128:(tt + 1) * 128],
                        pt.rearrange("p g (a b) -> p (g a) b", a=4))

                # qk projection for the whole chunk -> qkT bf16
                pq = pqv.tile([128, 512], F32, tag="pqv", name="pq")
                for ct in range(NCT):
                    nc.tensor.matmul(pq, wqkb[:, ct, :], xT[:, ct, t0:t1],
                                     start=(ct == 0), stop=(ct == NCT - 1))
                nc.vector.tensor_copy(qkT[:, t0:t1], pq)

                # kT rows -> base partition 0 (Pool/SWDGE, crosses partitions)
                nc.gpsimd.dma_start(kTsb[:, t0:t1], qkT[64:128, t0:t1])

                # v projection (natural layout) per tile -> v_sb
                for tt in range(4 * c, 4 * c + 4):
                    pv = pqv.tile([128, 512], F32, tag="pqv", name="pv")
                    for ct in range(NCT):
                        nc.tensor.matmul(pv[:, 0:H],
                                         xT[:, ct, tt * 128:(tt + 1) * 128],
                                         wvb[:, ct, :],
                                         start=(ct == 0), stop=(ct == NCT - 1))
                    nc.vector.tensor_copy(v_sb[:, tt, 0:H], pv[:, 0:H])

                sts = ST_WAVE[w]
                flush_pairs([sts[n:n + 2] for n in range(0, len(sts), 2)])

            # ---- diagonal tri-masks on Pool, ordered by exp completion
            for (i, j) in exp_order:
                k = j - 4 * i
                if k >= 0:
                    wt, h = wst_loc[(i, j)]
                    nc.gpsimd.tensor_mul(wt[:, h, k * 128:(k + 1) * 128],
                                         wt[:, h, k * 128:(k + 1) * 128], trib)

            # ---- output: po bursts (gated on the final wave anyway)
            done = {c: 0 for c in range(4)}
            ob = {}
            for c in range(4):
                obt = outp.tile([128, 4, H], F32, tag=f"ob{c}", name=f"ob{c}")
                ob[c] = obt
            PO_ORDER = [0, 1, 2, 3, 12, 13, 14, 15, 8, 9, 10, 11, 4, 5, 6, 7]
            for tt in PO_ORDER:
                i, tl = tt // 4, tt % 4
                js = [j for j in AVAIL if j <= tt]
                pp = pqv.tile([128, 512], F32, tag="pqv", name="pp")
                for n, j in enumerate(js):
                    wt, h = wst_loc[(i, j)]
                    nc.tensor.matmul(pp[:, 0:66],
                                     wt[:, h, tl * 128:(tl + 1) * 128],
                                     v_sb[:, j, 0:66],
                                     start=(n == 0), stop=(n == len(js) - 1))
                rec = recp.tile([128, 1], F32, tag="rec", name="rec")
                nc.vector.reciprocal(rec, pp[:, 64:65])
                nc.vector.tensor_scalar_mul(ob[i][:, tl, :], pp[:, 0:H], rec)
                nc.sync.dma_start(
                    out_d[tt * 128:(tt + 1) * 128, :], ob[i][:, tl, :])

    nc.compile()
    return nc


def kernel(x, Wq, Wk, Wv, trace=False):
    import ml_dtypes
    BF = ml_dtypes.bfloat16
    x = np.ascontiguousarray(np.asarray(x, dtype=np.float32))
    Wq = np.asarray(Wq, dtype=np.float32)
    Wk = np.asarray(Wk, dtype=np.float32)
    Wv = np.asarray(Wv, dtype=np.float32)

    if "nc" not in _CACHE:
        _CACHE["nc"] = build()
    nc = _CACHE["nc"]

    xb = np.ascontiguousarray(x.astype(BF))                       # [B, T, C]
    wqkb = np.ascontiguousarray(np.concatenate(
        [Wq.reshape(NCT, 128, H), Wk.reshape(NCT, 128, H)],
        axis=-1).transpose(1, 0, 2).astype(BF))                   # [128, 8, 128]
    wvb = np.ascontiguousarray(
        Wv.reshape(NCT, 128, H).transpose(1, 0, 2).astype(BF))    # [128, 8, 64]
    trib = np.triu(np.ones((128, 128), dtype=np.float32)).astype(BF)
    identb = np.eye(128, dtype=np.float32).astype(BF)

    in_maps = [
        {"xb": xb[b], "wqkb": wqkb, "wvb": wvb, "trib": trib, "identb": identb}
        for b in range(B)
    ]
    try:
        res = run_bass_kernel_spmd(nc, in_maps, core_ids=list(range(B)), trace=trace)
    except ModuleNotFoundError:
        res = run_bass_kernel_spmd(nc, in_maps, core_ids=list(range(B)))
    out = np.stack([r["out"] for r in res.results], axis=0)
    kernel.last_exec_time_ns = res.exec_time_ns
    kernel.last_results = res
    return out
